# revision 1
# baseline (speedup 1.0000x reference)
"""GAT layer (nn_GATLayer) on 8 Trainium2 NeuronCores.

Math (per batch b):
    h   = x @ W                      [N, D]
    s1  = h @ a1   (free-dim i)      [N]
    s2  = h @ a2   (partition j)     [N]
    e   = lrelu(s1_i + s2_j)  masked by adj[i, j], softmax over j
    out = attn @ h

Device formulation (per core = one batch element), in [p=j, f=i] layout:
    PT[j, i] = exp(0.2 * max(y, 5y)),  y = s1[i] + s2[j] + maskbias[j, i]
      (lrelu(x) = 0.2*max(5x, x); maskbias = 0 or -1e9 pre-lrelu, exp -> 0)
    numT[d, i] = sum_j h_cat[j, d] * PT[j, i],  h_cat = [h | ones]  (bf16)
    out[i, d]  = numT[d, i] / numT[64, i]

Sharding: data-parallel over batch B=8 across the 8 cores. Host prep:
x[b] transposed to xT [64, 2048]; maskbias = where(adj.T>0, 0, -1e9) bf16
(shared across cores).

Bacc's generate_event_semaphores handles the 1-wait-per-instruction HW
limit; tiles read by PE ops are still staged through ACT to keep wait
pressure low.
"""

import os
import sys

sys.path.insert(0, "/opt/trn_rl_repo")

import numpy as np
import ml_dtypes

B, N, DIN, DOUT = 8, 2048, 64, 64
NCORES = 8
PJ = 128              # j-tile partition size
NJT = N // PJ         # 16 j-tiles
FCH = 512             # psum bank chunk (fp32)
NCH = N // FCH        # 4 chunks of the free dim
NEG_BIG = -1.0e9
HCAT_STRIDE = 66      # 64 h cols + 1 ones col + 1 pad
EPI_GRP = 4           # epilogue transposes packed per psum bank tile

_GAT_OP = None
_COMPILED = None
LAST_RESULT = None    # BassKernelResults from the last run (for test.py)


def _register_gat_op():
    """Fused score op:  out = max(y, y*imm2), y = (in0 + s0) + in1.

    in0 = s1 broadcast [128, N] (f32), s0 = s2 per-partition [128, 1] (f32),
    in1 = maskbias tile [128, N] (bf16), imm2 = 5.0.
    """
    global _GAT_OP
    if _GAT_OP is not None:
        return _GAT_OP
    from concourse.dve_ops import (
        OPS,
        CUSTOM_DVE_SPECS,
        DveOp,
        _SUB_OPCODE_FOR_NAME,
    )
    from concourse.dve_spec import Spec, Src0, Src1, C0, C2, maxx, lower, _has_src1
    from concourse.dve_uop import DveOpSpec

    name = "GAT_SCORE_ANT"
    if name in _SUB_OPCODE_FOR_NAME:
        _GAT_OP = next(op for op in OPS if op.name == name)
        return _GAT_OP

    _y = (Src0 + C0) + Src1

    def _ref(in0, in1, s0, s1, imm2):
        y = (in0.astype(np.float32) + s0) + in1.astype(np.float32)
        return np.maximum(y, y * imm2).astype(np.float32)

    spec = Spec(body=maxx(_y, _y * C2), reference=_ref)
    row = max(_SUB_OPCODE_FOR_NAME.values()) + 1
    assert row < 0x20
    _SUB_OPCODE_FOR_NAME[name] = row
    shas = {}
    for ver in ("v3", "v4"):
        tmp = DveOpSpec(
            name=name, opcode=row, uops=lower(spec, ver=ver), rd1_en=_has_src1(spec)
        )
        shas[ver] = tmp.sha(ver)
    op = DveOp(name, spec, subdim=False, uops_sha=shas)
    OPS.append(op)
    CUSTOM_DVE_SPECS[name] = spec
    _GAT_OP = op
    return op


def _build_nc():
    """Build the Bass module (shared SPMD program for all 8 cores)."""
    from contextlib import ExitStack

    import concourse.bass as bass
    import concourse.tile as tile
    from concourse import bacc, masks, mybir

    gat_op = _register_gat_op()

    f32 = mybir.dt.float32
    bf16 = mybir.dt.bfloat16
    AF = mybir.ActivationFunctionType

    nc = bacc.Bacc("TRN2", target_bir_lowering=False, debug=False, num_devices=NCORES)

    use_f32r = bool(int(os.environ.get("GAT_F32R", "1")))
    xT = nc.dram_tensor("xt", [DIN, N], f32, kind="ExternalInput").ap()
    mb = nc.dram_tensor("maskbias", [N, N], bf16, kind="ExternalInput").ap()
    w = nc.dram_tensor("w", [DIN, DOUT], f32, kind="ExternalInput").ap()
    a1 = nc.dram_tensor("a1", [DOUT, 1], f32, kind="ExternalInput").ap()
    a2 = nc.dram_tensor("a2", [DOUT, 1], f32, kind="ExternalInput").ap()
    out = nc.dram_tensor("out", [N, DOUT], f32, kind="ExternalOutput").ap()

    with ExitStack() as ctx:
        tc = ctx.enter_context(tile.TileContext(nc))

        const = ctx.enter_context(tc.tile_pool(name="const", bufs=1))
        big = ctx.enter_context(tc.tile_pool(name="big", bufs=1))

        # ---- inputs to SBUF ----
        f32r = mybir.dt.float32r
        mmdt = f32r if use_f32r else f32

        # HWDGE queues, issued before the mask prefetches: each queue serves
        # these first, so xT lands ~2us in instead of queueing behind masks.
        w_dma = const.tile([DIN, DOUT], f32, tag="w0")
        nc.sync.dma_start(w_dma[:], w)
        a1_dma = const.tile([DOUT, 1], f32, tag="a10")
        nc.sync.dma_start(a1_dma[:], a1)
        a2_dma = const.tile([DOUT, 1], f32, tag="a20")
        nc.sync.dma_start(a2_dma[:], a2)
        xT_sb = const.tile([DIN, N], f32, tag="xt")
        # one dma_start: DMA bandwidth scales with partitions covered, so
        # splitting by rows would cut SBUF port width per transfer
        nc.sync.dma_start(xT_sb[:], xT)


        # ACT-staged copies: every tile a PE instruction reads is written by
        # the ACT engine (also performs the f32r rounding).
        w_sb = const.tile([DIN, DOUT], mmdt, tag="w")
        nc.scalar.copy(w_sb[:], w_dma[:])
        a1_sb = const.tile([DOUT, 1], f32, tag="a1")
        nc.scalar.copy(a1_sb[:], a1_dma[:])
        a2_sb = const.tile([DOUT, 1], f32, tag="a2")
        nc.scalar.copy(a2_sb[:], a2_dma[:])

        ident0 = const.tile([PJ, PJ], f32, tag="ident0")
        masks.make_identity(nc, ident0[:])
        ident = const.tile([PJ, PJ], f32, tag="ident")
        nc.scalar.copy(ident[:], ident0[:])

        ones_sb = const.tile([PJ, 1], bf16, tag="ones")
        nc.vector.memset(ones_sb[:], 1.0)

        # xT rounded to f32r on DVE (idle in the prologue window)
        if use_f32r:
            xTr = const.tile([DIN, N], f32r, tag="xtr")
            nc.vector.tensor_copy(xTr[:], xT_sb[:])
        else:
            xTr = xT_sb

        hT_sb = big.tile([DIN, N], f32, tag="ht")      # h^T
        s1b_sb = big.tile([PJ, N], f32, tag="s1b")     # s1 broadcast to 128 rows
        s2_all = big.tile([PJ, NJT], f32, tag="s2")    # s2, col jt = j-tile chunk
        hcat = big.tile([PJ, NJT * HCAT_STRIDE], bf16, tag="hcat")  # [h | 1]

        # ones columns of h_cat via one strided ACT copy
        hcat3 = hcat[:].rearrange("p (t s) -> p t s", s=HCAT_STRIDE)
        nc.scalar.copy(
            hcat3[:, :, DOUT : DOUT + 1],
            ones_sb[:].broadcast_to([PJ, NJT])[:, :, None],
        )

        # ---- wa1 = W @ a1, replicated: lets s1b come straight from xT ----
        # s1[i] = sum_d x[i,d] (W@a1)[d], so s1b = wa1rep.T @ xT needs no h.
        with tc.tile_pool(name="wt_psum", bufs=1, space="PSUM") as wtpool:
            wt_ps = wtpool.tile([DOUT, DIN], f32, tag="wt_ps")
            nc.tensor.transpose(wt_ps[:], w_dma[:], ident0[:DIN, :DIN])
            wt_sb = const.tile([DOUT, DIN], f32, tag="wt")
            nc.scalar.copy(wt_sb[:], wt_ps[:])
            wa1_ps = wtpool.tile([DIN, 1], f32, tag="wa1_ps")
            nc.tensor.matmul(wa1_ps[:], wt_sb[:], a1_sb[:], start=True, stop=True)
            wa1rep = const.tile([DIN, PJ], mmdt, tag="wa1rep")
            nc.scalar.copy(wa1rep[:], wa1_ps[:].broadcast_to([DIN, PJ]))
            # wa2 = W @ a2: lets s2 come straight from xT as well
            # (fp32 operands: f32r is rejected as a stationary operand here)
            wa2_ps = wtpool.tile([DIN, 1], f32, tag="wa2_ps")
            nc.tensor.matmul(wa2_ps[:], wt_sb[:], a2_sb[:], start=True, stop=True)
            wa2_sb = const.tile([DIN, 1], f32, tag="wa2")
            nc.scalar.copy(wa2_sb[:], wa2_ps[:])

        # ---- prologue: h^T and s1b, back-to-back on PE (both read only xT)
        with tc.tile_pool(name="pro_psum", bufs=1, space="PSUM") as ppool, \
             tc.tile_pool(name="pro2_psum", bufs=1, space="PSUM") as ppool2:
            hT_ps = ppool.tile([DIN, N], f32, tag="ht_ps")
            s1b_ps = ppool2.tile([PJ, N], f32, tag="s1b_ps")
            for c in range(NCH):
                sl = slice(c * FCH, (c + 1) * FCH)
                nc.tensor.matmul(
                    hT_ps[:, sl], w_sb[:], xTr[:, sl], start=True, stop=True
                )
                nc.scalar.copy(hT_sb[:, sl], hT_ps[:, sl])
                nc.tensor.matmul(
                    s1b_ps[:, sl], wa1rep[:], xTr[:, sl], start=True, stop=True
                )
                nc.vector.tensor_copy(s1b_sb[:, sl], s1b_ps[:, sl])

        with tc.tile_pool(name="s2_psum", bufs=2, space="PSUM") as spool, \
             tc.tile_pool(name="htr_psum", bufs=2, space="PSUM") as ppool3:
            # s2 chunks straight from xT: lhsT = xTr chunk, rhs = wa2 -> [128,1]
            # two groups of 8 in separate psum banks with ONE drain copy each
            # (per-chunk drains would ping-pong PE<->ACT on a single bank)
            for g in range(2):
                s2_ps = spool.tile([PJ, 8], f32, tag="s2_ps")
                for k in range(8):
                    jt = g * 8 + k
                    jsl = slice(jt * PJ, (jt + 1) * PJ)
                    nc.tensor.matmul(
                        s2_ps[:, k : k + 1], xT_sb[:, jsl], wa2_sb[:],
                        start=True, stop=True,
                    )
                nc.scalar.copy(s2_all[:, g * 8 : (g + 1) * 8], s2_ps[:])

            # h tiles: PE-transpose hT chunks, pack 8 per psum bank, cast bf16
            for half in range(2):
                htr_ps = ppool3.tile([PJ, 8 * DOUT], f32, tag="htr")
                for k in range(8):
                    jt = half * 8 + k
                    jsl = slice(jt * PJ, (jt + 1) * PJ)
                    nc.tensor.transpose(
                        htr_ps[:, k * DOUT : (k + 1) * DOUT],
                        hT_sb[:, jsl],
                        ident[:DIN, :DIN],
                    )
                dst = hcat3[:, half * 8 : (half + 1) * 8, :DOUT]
                src = htr_ps[:].rearrange("p (t s) -> p t s", s=DOUT)
                nc.scalar.copy(dst, src)

        # ---- main loop over j-tiles ----
        mpool = ctx.enter_context(tc.tile_pool(name="mask", bufs=6))
        tpool = ctx.enter_context(tc.tile_pool(name="scores", bufs=3))
        ppool_e = ctx.enter_context(tc.tile_pool(name="probs", bufs=3))
        num_pool = ctx.enter_context(
            tc.tile_pool(name="num_psum", bufs=1, space="PSUM")
        )

        numT_ps = num_pool.tile([DOUT + 1, N], f32, tag="numt")

        for jt in range(NJT):
            mb_sb = mpool.tile([PJ, N], bf16, tag="mb")
            if jt < 6:
                # WAR gate: each prefetch-window mask DMA overwrites a probe
                # byte that depends on xTr, so the whole mask stream waits
                # until xT has fully landed -- the concurrent HWDGE queues
                # would otherwise steal ~4/5 of the bandwidth from it.
                nc.vector.tensor_copy(mb_sb[0:1, 0:1], xTr[0:1, 0:1])
            # schedule-time floor: behind the input loads, in jt order
            with tc.tile_wait_until(0.002 + 0.0001 * jt):
                nc.sync.dma_start(mb_sb[:], mb[jt * PJ : (jt + 1) * PJ, :])

            t_sb = tpool.tile([PJ, N], f32, tag="t")
            nc.vector._custom_dve(
                gat_op,
                out=t_sb[:],
                in0=s1b_sb[:],
                in1=mb_sb[:],
                s0=s2_all[:, jt : jt + 1],
                s1=0.0,
                imm2=5.0,
            )

            p_sb = ppool_e.tile([PJ, N], bf16, tag="p")
            nc.scalar.activation(p_sb[:], t_sb[:], AF.Exp, scale=0.2)

            lhsT = hcat[:, jt * HCAT_STRIDE : jt * HCAT_STRIDE + DOUT + 1]
            for c in range(NCH):
                sl = slice(c * FCH, (c + 1) * FCH)
                nc.tensor.matmul(
                    numT_ps[:, sl], lhsT, p_sb[:, sl],
                    start=(jt == 0), stop=(jt == NJT - 1),
                )

        # ---- epilogue: transpose numT, divide by row-sums, store ----
        epool = ctx.enter_context(tc.tile_pool(name="epi", bufs=2))
        etr_pool = ctx.enter_context(
            tc.tile_pool(name="epi_psum", bufs=2, space="PSUM")
        )
        out_pool = ctx.enter_context(tc.tile_pool(name="out", bufs=1))

        numT_sb = big.tile([DOUT + 1, N], f32, tag="numt_sb")
        nc.scalar.copy(numT_sb[:], numT_ps[:])

        out_sb = out_pool.tile([PJ, NJT * DOUT], f32, tag="out")
        GW = EPI_GRP * (DOUT + 1)  # grouped transpose width per psum tile
        for g in range(NJT // EPI_GRP):
            tr_ps = etr_pool.tile([PJ, GW], f32, tag="tr")
            for k in range(EPI_GRP):
                it = g * EPI_GRP + k
                isl = slice(it * PJ, (it + 1) * PJ)
                nc.tensor.transpose(
                    tr_ps[:, k * (DOUT + 1) : (k + 1) * (DOUT + 1)],
                    numT_sb[:, isl],
                    ident[: DOUT + 1, : DOUT + 1],
                )
            # single ACT drain per group keeps the PSUM slot reader on ACT
            tr_sb = epool.tile([PJ, GW], f32, tag="tr_sb")
            nc.scalar.copy(tr_sb[:], tr_ps[:])

            tr3 = tr_sb[:].rearrange("p (k s) -> p k s", s=DOUT + 1)
            recip = epool.tile([PJ, EPI_GRP], f32, tag="recip")
            nc.vector.reciprocal(recip[:], tr3[:, :, DOUT])
            for k in range(EPI_GRP):
                it = g * EPI_GRP + k
                nc.vector.tensor_scalar_mul(
                    out_sb[:, it * DOUT : (it + 1) * DOUT],
                    tr3[:, k, :DOUT],
                    recip[:, k : k + 1],
                )

        out_3d = out.rearrange("(t p) d -> p t d", p=PJ)
        nc.sync.dma_start(out_3d, out_sb[:].rearrange("p (t d) -> p t d", d=DOUT))

    nc.compile()
    return nc


def _prep_inputs(x, adj, W, a):
    xT = np.ascontiguousarray(np.transpose(x, (0, 2, 1)), dtype=np.float32)
    mask_bias = np.where(adj.T > 0, np.float32(0.0), np.float32(NEG_BIG)).astype(
        ml_dtypes.bfloat16
    )
    a = np.asarray(a, dtype=np.float32)
    a1 = np.ascontiguousarray(a[:DOUT].reshape(DOUT, 1))
    a2 = np.ascontiguousarray(a[DOUT:].reshape(DOUT, 1))
    W = np.ascontiguousarray(np.asarray(W, dtype=np.float32))
    in_maps = []
    for b in range(NCORES):
        in_maps.append(
            {
                "xt": xT[b],
                "maskbias": mask_bias,
                "w": W,
                "a1": a1,
                "a2": a2,
            }
        )
    return in_maps


def kernel(x, adj, W, a):
    global _COMPILED, LAST_RESULT
    from concourse import bass_utils

    x = np.asarray(x)
    adj = np.asarray(adj)
    assert x.shape == (B, N, DIN) and adj.shape == (N, N)

    if _COMPILED is None:
        _COMPILED = _build_nc()
    nc = _COMPILED

    in_maps = _prep_inputs(x, adj, W, a)
    res = bass_utils.run_bass_kernel_spmd(
        nc,
        in_maps,
        core_ids=list(range(NCORES)),
        trace=bool(int(os.environ.get("GAT_TRACE", "0"))),
    )
    LAST_RESULT = res
    out = np.stack([res.results[c]["out"] for c in range(NCORES)], axis=0)
    return out.astype(np.float32)



# revision 5
# speedup vs baseline: 1.0736x; 1.0736x over previous
"""GAT layer (nn_GATLayer) on 8 Trainium2 NeuronCores.

Math (per batch b):
    h   = x @ W                      [N, D]
    s1  = h @ a1   (free-dim i)      [N]
    s2  = h @ a2   (partition j)     [N]
    e   = lrelu(s1_i + s2_j)  masked by adj[i, j], softmax over j
    out = attn @ h

Device formulation (per core = one batch element), in [p=j, f=i] layout:
    t[j, i]  = select(A[j, i] > 0, max(y, 5y), -1e9),  y = s1[i] + s2[j]
               (custom DVE op; A is the uint8 adjacency -> 4MB/core DMA)
    p[j, i]  = exp(0.2 * t)                      (ACT, bf16 out)
    numT[d, i] = sum_j h_cat[j, d] * p[j, i],    h_cat = [h | ones]  (bf16)
    out[i, d]  = numT[d, i] / numT[64, i]

Sharding: data-parallel over batch B=8 across the 8 cores. Host prep:
x[b] transposed to xT [64, 2048] (split in 4 column chunks for parallel
queue DMA); mask = adj.T as uint8 (shared across cores).

Prologue keeps the loop-critical path short: s1b comes straight from xT
via wa1 = W@a1 broadcast (f32r matmuls), and each j-tile's h/s2 come from
ONE matmul with rhs = [W | W@a2] (bf16) against the xT chunk (f32r
bitcast), drained per group-of-4 into write-once hcat regions.
"""

import os
import sys

sys.path.insert(0, "/opt/trn_rl_repo")

import numpy as np

B, N, DIN, DOUT = 8, 2048, 64, 64
NCORES = 8
PJ = 128              # j-tile partition size
NJT = N // PJ         # 16 j-tiles
FCH = 512             # psum bank chunk (fp32)
NCH = N // FCH        # 4 chunks of the free dim
XCH = 4               # xT column chunks (parallel DMA queues)
NEG_BIG = -1.0e9
HCG = 4               # h/s2 tiles per psum group / hcat region
HCS = 66              # hcat stride: 64 h cols + 1 ones col + 1 pad
EPI_GRP = 4           # epilogue transposes packed per psum bank tile

_GAT_OP = None
_COMPILED = None
LAST_RESULT = None    # BassKernelResults from the last run (for test.py)


def _register_gat_op():
    """Fused score op: out = select(Src1 > 0, max(y, y*imm2), C1), y = Src0+C0.

    in0 = s1 broadcast [128, N] (f32), s0 = s2 per-partition [128, 1] (f32),
    in1 = adjacency tile [128, N] (uint8 0/1), s1 = -1e9, imm2 = 5.0.
    lrelu(x) = 0.2*max(5x, x); exp(0.2 * -1e9) -> 0 for masked entries.
    """
    global _GAT_OP
    if _GAT_OP is not None:
        return _GAT_OP
    from concourse.dve_ops import (
        OPS,
        CUSTOM_DVE_SPECS,
        DveOp,
        _SUB_OPCODE_FOR_NAME,
    )
    from concourse.dve_spec import (
        Spec, Src0, Src1, C0, C1, C2, Zero, maxx, select, lower, _has_src1,
    )
    from concourse.dve_uop import DveOpSpec

    name = "GAT_SCORE_U8_ANT"
    if name in _SUB_OPCODE_FOR_NAME:
        _GAT_OP = next(op for op in OPS if op.name == name)
        return _GAT_OP

    _y = Src0 + C0
    body = select(Src1 > Zero, maxx(_y, _y * C2), C1)

    def _ref(in0, in1, s0, s1, imm2):
        y = in0.astype(np.float32) + s0
        t = np.maximum(y, y * imm2)
        return np.where(in1.astype(np.float32) > 0.0, t, s1).astype(np.float32)

    spec = Spec(body=body, reference=_ref)
    row = max(_SUB_OPCODE_FOR_NAME.values()) + 1
    assert row < 0x20
    _SUB_OPCODE_FOR_NAME[name] = row
    shas = {}
    for ver in ("v3", "v4"):
        tmp = DveOpSpec(
            name=name, opcode=row, uops=lower(spec, ver=ver), rd1_en=_has_src1(spec)
        )
        shas[ver] = tmp.sha(ver)
    op = DveOp(name, spec, subdim=False, uops_sha=shas)
    OPS.append(op)
    CUSTOM_DVE_SPECS[name] = spec
    _GAT_OP = op
    return op


def _build_nc():
    """Build the Bass module (shared SPMD program for all 8 cores)."""
    from contextlib import ExitStack

    import concourse.bass as bass
    import concourse.tile as tile
    from concourse import bacc, masks, mybir

    gat_op = _register_gat_op()

    f32 = mybir.dt.float32
    f32r = mybir.dt.float32r
    bf16 = mybir.dt.bfloat16
    u8 = mybir.dt.uint8
    AF = mybir.ActivationFunctionType

    nc = bacc.Bacc("TRN2", target_bir_lowering=False, debug=False, num_devices=NCORES)

    xT = nc.dram_tensor("xt", [DIN, N], f32, kind="ExternalInput").ap()
    mk = nc.dram_tensor("mask", [N, N], u8, kind="ExternalInput").ap()
    w = nc.dram_tensor("w", [DIN, DOUT], f32, kind="ExternalInput").ap()
    a1 = nc.dram_tensor("a1", [DOUT, 1], f32, kind="ExternalInput").ap()
    a2 = nc.dram_tensor("a2", [DOUT, 1], f32, kind="ExternalInput").ap()
    out = nc.dram_tensor("out", [N, DOUT], f32, kind="ExternalOutput").ap()

    with ExitStack() as ctx:
        tc = ctx.enter_context(tile.TileContext(nc))

        const = ctx.enter_context(tc.tile_pool(name="const", bufs=1))
        big = ctx.enter_context(tc.tile_pool(name="big", bufs=1))

        # ---- input DMAs: w/a1/a2 first (tiny, unblock the wa chain), then
        # xT in 4 column chunks (4 parallel queues), then the mask stream.
        w_dma = const.tile([DIN, DOUT], f32, tag="w0")
        nc.sync.dma_start(w_dma[:], w)
        a1_dma = const.tile([DOUT, 1], f32, tag="a10")
        nc.sync.dma_start(a1_dma[:], a1)
        a2_dma = const.tile([DOUT, 1], f32, tag="a20")
        nc.sync.dma_start(a2_dma[:], a2)
        xt = []
        for c in range(XCH):
            t = const.tile([DIN, N // XCH], f32, tag=f"xt{c}")
            nc.sync.dma_start(t[:], xT[:, c * (N // XCH) : (c + 1) * (N // XCH)])
            xt.append(t)

        ident0 = const.tile([PJ, PJ], f32, tag="ident0")
        masks.make_identity(nc, ident0[:])
        ident = const.tile([PJ, PJ], f32, tag="ident")
        nc.scalar.copy(ident[:], ident0[:])

        ones_sb = const.tile([PJ, 1], bf16, tag="ones")
        nc.vector.memset(ones_sb[:], 1.0)

        # ---- wa1 = W @ a1 (replicated f32r), wa2 = W @ a2; wrhs = [W | wa2]
        a1_sb = const.tile([DOUT, 1], f32, tag="a1")
        nc.scalar.copy(a1_sb[:], a1_dma[:])
        a2_sb = const.tile([DOUT, 1], f32, tag="a2")
        nc.scalar.copy(a2_sb[:], a2_dma[:])

        wrhs = const.tile([DIN, DOUT + 1], bf16, tag="wrhs")
        nc.scalar.copy(wrhs[:, :DOUT], w_dma[:])

        with tc.tile_pool(name="wt_psum", bufs=1, space="PSUM") as wtpool:
            wt_ps = wtpool.tile([DOUT, DIN], f32, tag="wt_ps")
            nc.tensor.transpose(wt_ps[:], w_dma[:], ident0[:DIN, :DIN])
            wt_sb = const.tile([DOUT, DIN], f32, tag="wt")
            nc.scalar.copy(wt_sb[:], wt_ps[:])
            wa1_ps = wtpool.tile([DIN, 1], f32, tag="wa1_ps")
            nc.tensor.matmul(wa1_ps[:], wt_sb[:], a1_sb[:], start=True, stop=True)
            wa1rep = const.tile([DIN, PJ], bf16, tag="wa1rep")
            nc.scalar.copy(wa1rep[:], wa1_ps[:].broadcast_to([DIN, PJ]))
            wa2_ps = wtpool.tile([DIN, 1], f32, tag="wa2_ps")
            nc.tensor.matmul(wa2_ps[:], wt_sb[:], a2_sb[:], start=True, stop=True)
            nc.scalar.copy(wrhs[:, DOUT : DOUT + 1], wa2_ps[:])

        # ---- xt chunks cast to bf16 (DVE): feeds s1b and the h/s2 matmuls
        xtb = []
        for c in range(XCH):
            t = const.tile([DIN, N // XCH], bf16, name=f"xtb{c}", tag=f"xtb{c}")
            nc.vector.tensor_copy(t[:], xt[c][:])
            xtb.append(t)

        # ---- s1b = wa1rep.T @ xT : [128, N] f32, straight from xT chunks
        s1b_sb = big.tile([PJ, N], f32, tag="s1b")
        with tc.tile_pool(name="s1b_psum", bufs=1, space="PSUM") as spool:
            s1b_ps = spool.tile([PJ, N], f32, tag="s1b_ps")
            for c in range(NCH):
                sl = slice(c * FCH, (c + 1) * FCH)
                nc.tensor.matmul(
                    s1b_ps[:, sl], wa1rep[:], xtb[c][:],
                    start=True, stop=True,
                )
                nc.scalar.copy(s1b_sb[:, sl], s1b_ps[:, sl])

        # ---- hcat regions (write-once, 4 tiles each) with ones columns
        hcat = [
            big.tile([PJ, HCG * HCS], bf16, name=f"hcat{g}", tag=f"hcat{g}")
            for g in range(NJT // HCG)
        ]
        s2g = [
            big.tile([PJ, HCG], f32, name=f"s2g{g}", tag=f"s2g{g}")
            for g in range(NJT // HCG)
        ]
        for g in range(NJT // HCG):
            h3 = hcat[g][:].rearrange("p (t s) -> p t s", s=HCS)
            nc.scalar.copy(
                h3[:, :, DOUT : DOUT + 1],
                ones_sb[:].broadcast_to([PJ, HCG])[:, :, None],
            )

        # ---- main pools ----
        mpool = ctx.enter_context(tc.tile_pool(name="mask", bufs=6))
        tpool = ctx.enter_context(tc.tile_pool(name="scores", bufs=3))
        ppool_e = ctx.enter_context(tc.tile_pool(name="probs", bufs=3))
        hpool = ctx.enter_context(tc.tile_pool(name="h_psum", bufs=2, space="PSUM"))
        num_pool = ctx.enter_context(
            tc.tile_pool(name="num_psum", bufs=1, space="PSUM")
        )
        numT_ps = num_pool.tile([DOUT + 1, N], f32, tag="numt")

        # ---- h/s2 tiles: per group of 4, one matmul per tile into a shared
        # psum bank tile, then ONE strided ACT drain for h and one for s2.
        def emit_hgroup(g):
            h_ps = hpool.tile([PJ, HCG * (DOUT + 1)], f32, tag="hps")
            for k in range(HCG):
                jt = g * HCG + k
                c, off = jt // XCH, (jt % XCH) * PJ
                nc.tensor.matmul(
                    h_ps[:, k * (DOUT + 1) : (k + 1) * (DOUT + 1)],
                    xtb[c][:, off : off + PJ],
                    wrhs[:],
                    start=True, stop=True,
                )
            h4 = h_ps[:].rearrange("p (t s) -> p t s", s=DOUT + 1)
            hc3 = hcat[g][:].rearrange("p (t s) -> p t s", s=HCS)
            nc.scalar.copy(hc3[:, :, :DOUT], h4[:, :, :DOUT])
            nc.scalar.copy(s2g[g][:], h4[:, :, DOUT])

        emit_hgroup(0)
        emit_hgroup(1)

        # ---- main loop over j-tiles ----
        for jt in range(NJT):
            g, k = jt // HCG, jt % HCG
            if jt in (HCG, 2 * HCG) :
                emit_hgroup(g + 1)

            mb = mpool.tile([PJ, N], u8, tag="mb")
            if jt < 8:
                # WAR gate: the probe byte depends on the last xT chunk, so
                # the prefetch-window mask DMAs wait until xT has landed.
                nc.vector.tensor_copy(mb[0:1, 0:1], xt[XCH - 1][0:1, 0:1])
            with tc.tile_wait_until(0.0012 + 0.0001 * jt):
                nc.sync.dma_start(mb[:], mk[jt * PJ : (jt + 1) * PJ, :])

            t_sb = tpool.tile([PJ, N], f32, tag="t")
            nc.vector._custom_dve(
                gat_op,
                out=t_sb[:],
                in0=s1b_sb[:],
                in1=mb[:],
                s0=s2g[g][:, k : k + 1],
                s1=NEG_BIG,
                imm2=5.0,
            )

            p_sb = ppool_e.tile([PJ, N], bf16, tag="p")
            nc.scalar.activation(p_sb[:], t_sb[:], AF.Exp, scale=0.2)

            lhsT = hcat[g][:, k * HCS : k * HCS + DOUT + 1]
            for c in range(NCH):
                sl = slice(c * FCH, (c + 1) * FCH)
                nc.tensor.matmul(
                    numT_ps[:, sl], lhsT, p_sb[:, sl],
                    start=(jt == 0), stop=(jt == NJT - 1),
                )

        # ---- epilogue: per 512-chunk: drain, transpose, divide, store ----
        epool = ctx.enter_context(tc.tile_pool(name="epi", bufs=2))
        etr_pool = ctx.enter_context(
            tc.tile_pool(name="epi_psum", bufs=2, space="PSUM")
        )
        out_pool = ctx.enter_context(tc.tile_pool(name="out", bufs=1))

        out_sb = out_pool.tile([PJ, NJT * DOUT], f32, tag="out")
        out_3d = out.rearrange("(t p) d -> p t d", p=PJ)
        GW = EPI_GRP * (DOUT + 1)
        for g in range(NJT // EPI_GRP):
            csl = slice(g * FCH, (g + 1) * FCH)
            numc = epool.tile([DOUT + 1, FCH], f32, tag="numc")
            nc.scalar.copy(numc[:], numT_ps[:, csl])

            tr_ps = etr_pool.tile([PJ, GW], f32, tag="tr")
            for k in range(EPI_GRP):
                isl = slice(k * PJ, (k + 1) * PJ)
                nc.tensor.transpose(
                    tr_ps[:, k * (DOUT + 1) : (k + 1) * (DOUT + 1)],
                    numc[:, isl],
                    ident[: DOUT + 1, : DOUT + 1],
                )
            tr_sb = epool.tile([PJ, GW], f32, tag="tr_sb")
            nc.scalar.copy(tr_sb[:], tr_ps[:])

            tr3 = tr_sb[:].rearrange("p (k s) -> p k s", s=DOUT + 1)
            recip = epool.tile([PJ, EPI_GRP], f32, tag="recip")
            nc.vector.reciprocal(recip[:], tr3[:, :, DOUT])
            for k in range(EPI_GRP):
                it = g * EPI_GRP + k
                nc.vector.tensor_scalar_mul(
                    out_sb[:, it * DOUT : (it + 1) * DOUT],
                    tr3[:, k, :DOUT],
                    recip[:, k : k + 1],
                )
            nc.sync.dma_start(
                out_3d[:, g * EPI_GRP : (g + 1) * EPI_GRP, :],
                out_sb[:, g * EPI_GRP * DOUT : (g + 1) * EPI_GRP * DOUT].rearrange(
                    "p (t d) -> p t d", d=DOUT
                ),
            )

    nc.compile()
    return nc


def _prep_inputs(x, adj, W, a):
    xT = np.ascontiguousarray(np.transpose(x, (0, 2, 1)), dtype=np.float32)
    mask = np.ascontiguousarray(adj.T.astype(np.uint8))
    a = np.asarray(a, dtype=np.float32)
    a1 = np.ascontiguousarray(a[:DOUT].reshape(DOUT, 1))
    a2 = np.ascontiguousarray(a[DOUT:].reshape(DOUT, 1))
    W = np.ascontiguousarray(np.asarray(W, dtype=np.float32))
    in_maps = []
    for b in range(NCORES):
        in_maps.append(
            {
                "xt": xT[b],
                "mask": mask,
                "w": W,
                "a1": a1,
                "a2": a2,
            }
        )
    return in_maps


def kernel(x, adj, W, a):
    global _COMPILED, LAST_RESULT
    from concourse import bass_utils

    x = np.asarray(x)
    adj = np.asarray(adj)
    assert x.shape == (B, N, DIN) and adj.shape == (N, N)

    if _COMPILED is None:
        _COMPILED = _build_nc()
    nc = _COMPILED

    in_maps = _prep_inputs(x, adj, W, a)
    res = bass_utils.run_bass_kernel_spmd(
        nc,
        in_maps,
        core_ids=list(range(NCORES)),
        trace=bool(int(os.environ.get("GAT_TRACE", "0"))),
    )
    LAST_RESULT = res
    out = np.stack([res.results[c]["out"] for c in range(NCORES)], axis=0)
    return out.astype(np.float32)


# revision 6
# speedup vs baseline: 1.1593x; 1.0798x over previous
"""GAT layer (nn_GATLayer) on 8 Trainium2 NeuronCores.

Math (per batch b):
    h   = x @ W                      [N, D]
    s1  = h @ a1   (free-dim i)      [N]
    s2  = h @ a2   (partition j)     [N]
    e   = lrelu(s1_i + s2_j)  masked by adj[i, j], softmax over j
    out = attn @ h

Device formulation (per core = one batch element), in [p=j, f=i] layout:
    t[j, i]  = select(A[j, i] > 0, max(y, 5y), -1e9),  y = s1[i] + s2[j]
               (custom DVE op; A is the uint8 adjacency -> 4MB/core DMA)
    p[j, i]  = exp(0.2 * t)                      (ACT, bf16 out)
    numT[d, i] = sum_j h_cat[j, d] * p[j, i],    h_cat = [h | ones]  (bf16)
    out[i, d]  = numT[d, i] / numT[64, i]

Sharding: data-parallel over batch B=8 across the 8 cores. Host prep:
x[b] transposed to xT [64, 2048] (split in 4 column chunks for parallel
queue DMA); mask = adj.T as uint8 (shared across cores).

Prologue keeps the loop-critical path short: s1b comes straight from xT
via wa1 = W@a1 broadcast (f32r matmuls), and each j-tile's h/s2 come from
ONE matmul with rhs = [W | W@a2] (bf16) against the xT chunk (f32r
bitcast), drained per group-of-4 into write-once hcat regions.
"""

import os
import sys

sys.path.insert(0, "/opt/trn_rl_repo")

import numpy as np

B, N, DIN, DOUT = 8, 2048, 64, 64
NCORES = 8
PJ = 128              # j-tile partition size
NJT = N // PJ         # 16 j-tiles
FCH = 512             # psum bank chunk (fp32)
NCH = N // FCH        # 4 chunks of the free dim
XCH = 4               # xT column chunks (parallel DMA queues)
NEG_BIG = -1.0e9
HCG = 4               # h/s2 tiles per psum group / hcat region
HCS = 66              # hcat stride: 64 h cols + 1 ones col + 1 pad
EPI_GRP = 4           # epilogue transposes packed per psum bank tile

_GAT_OP = None
_COMPILED = None
LAST_RESULT = None    # BassKernelResults from the last run (for test.py)


def _register_gat_op():
    """Fused score op: out = select(Src1 > 0, max(y, y*imm2), C1), y = Src0+C0.

    in0 = s1 broadcast [128, N] (f32), s0 = s2 per-partition [128, 1] (f32),
    in1 = adjacency tile [128, N] (uint8 0/1), s1 = -1e9, imm2 = 5.0.
    lrelu(x) = 0.2*max(5x, x); exp(0.2 * -1e9) -> 0 for masked entries.
    """
    global _GAT_OP
    if _GAT_OP is not None:
        return _GAT_OP
    from concourse.dve_ops import (
        OPS,
        CUSTOM_DVE_SPECS,
        DveOp,
        _SUB_OPCODE_FOR_NAME,
    )
    from concourse.dve_spec import (
        Spec, Src0, Src1, C0, C1, C2, Zero, maxx, select, lower, _has_src1,
    )
    from concourse.dve_uop import DveOpSpec

    name = "GAT_SCORE_U8_ANT"
    if name in _SUB_OPCODE_FOR_NAME:
        _GAT_OP = next(op for op in OPS if op.name == name)
        return _GAT_OP

    _y = Src0 + C0
    body = select(Src1 > Zero, maxx(_y, _y * C2), C1)

    def _ref(in0, in1, s0, s1, imm2):
        y = in0.astype(np.float32) + s0
        t = np.maximum(y, y * imm2)
        return np.where(in1.astype(np.float32) > 0.0, t, s1).astype(np.float32)

    spec = Spec(body=body, reference=_ref)
    row = max(_SUB_OPCODE_FOR_NAME.values()) + 1
    assert row < 0x20
    _SUB_OPCODE_FOR_NAME[name] = row
    shas = {}
    for ver in ("v3", "v4"):
        tmp = DveOpSpec(
            name=name, opcode=row, uops=lower(spec, ver=ver), rd1_en=_has_src1(spec)
        )
        shas[ver] = tmp.sha(ver)
    op = DveOp(name, spec, subdim=False, uops_sha=shas)
    OPS.append(op)
    CUSTOM_DVE_SPECS[name] = spec
    _GAT_OP = op
    return op


def _build_nc():
    """Build the Bass module (shared SPMD program for all 8 cores)."""
    from contextlib import ExitStack

    import concourse.bass as bass
    import concourse.tile as tile
    from concourse import bacc, masks, mybir

    gat_op = _register_gat_op()

    f32 = mybir.dt.float32
    f32r = mybir.dt.float32r
    bf16 = mybir.dt.bfloat16
    u8 = mybir.dt.uint8
    AF = mybir.ActivationFunctionType

    nc = bacc.Bacc("TRN2", target_bir_lowering=False, debug=False, num_devices=NCORES)

    xT = nc.dram_tensor("xt", [DIN, N], f32, kind="ExternalInput").ap()
    mk = nc.dram_tensor("mask", [N, N], u8, kind="ExternalInput").ap()
    w = nc.dram_tensor("w", [DIN, DOUT], f32, kind="ExternalInput").ap()
    a1 = nc.dram_tensor("a1", [DOUT, 1], f32, kind="ExternalInput").ap()
    a2 = nc.dram_tensor("a2", [DOUT, 1], f32, kind="ExternalInput").ap()
    out = nc.dram_tensor("out", [N, DOUT], f32, kind="ExternalOutput").ap()

    with ExitStack() as ctx:
        tc = ctx.enter_context(tile.TileContext(nc))

        const = ctx.enter_context(tc.tile_pool(name="const", bufs=1))
        big = ctx.enter_context(tc.tile_pool(name="big", bufs=1))

        # ---- input DMAs: xT chunks first on sync queues (critical path);
        # w/a1/a2 dispatched from the idle gpsimd engine in parallel.
        xt = []
        for c in range(XCH):
            t = const.tile([DIN, N // XCH], f32, name=f"xt{c}", tag=f"xt{c}")
            nc.sync.dma_start(t[:], xT[:, c * (N // XCH) : (c + 1) * (N // XCH)])
            xt.append(t)
        w_dma = const.tile([DIN, DOUT], f32, tag="w0")
        nc.gpsimd.dma_start(w_dma[:], w)
        a1_dma = const.tile([DOUT, 1], f32, tag="a10")
        nc.gpsimd.dma_start(a1_dma[:], a1)
        a2_dma = const.tile([DOUT, 1], f32, tag="a20")
        nc.gpsimd.dma_start(a2_dma[:], a2)

        ident0 = const.tile([PJ, PJ], f32, tag="ident0")
        masks.make_identity(nc, ident0[:])
        ident = const.tile([PJ, PJ], f32, tag="ident")
        nc.scalar.copy(ident[:], ident0[:])

        ones_sb = const.tile([PJ, 1], bf16, tag="ones")
        nc.vector.memset(ones_sb[:], 1.0)

        # ---- wa1 = W @ a1 (replicated f32r), wa2 = W @ a2; wrhs = [W | wa2]
        a1_sb = const.tile([DOUT, 1], f32, tag="a1")
        nc.scalar.copy(a1_sb[:], a1_dma[:])
        a2_sb = const.tile([DOUT, 1], f32, tag="a2")
        nc.scalar.copy(a2_sb[:], a2_dma[:])

        wrhs = const.tile([DIN, DOUT + 1], bf16, tag="wrhs")
        nc.scalar.copy(wrhs[:, :DOUT], w_dma[:])

        with tc.tile_pool(name="wt_psum", bufs=1, space="PSUM") as wtpool:
            wt_ps = wtpool.tile([DOUT, DIN], f32, tag="wt_ps")
            nc.tensor.transpose(wt_ps[:], w_dma[:], ident0[:DIN, :DIN])
            wt_sb = const.tile([DOUT, DIN], f32, tag="wt")
            nc.scalar.copy(wt_sb[:], wt_ps[:])
            wa1_ps = wtpool.tile([DIN, 1], f32, tag="wa1_ps")
            nc.tensor.matmul(wa1_ps[:], wt_sb[:], a1_sb[:], start=True, stop=True)
            wa1rep = const.tile([DIN, PJ], bf16, tag="wa1rep")
            nc.scalar.copy(wa1rep[:], wa1_ps[:].broadcast_to([DIN, PJ]))
            wa2_ps = wtpool.tile([DIN, 1], f32, tag="wa2_ps")
            nc.tensor.matmul(wa2_ps[:], wt_sb[:], a2_sb[:], start=True, stop=True)
            nc.scalar.copy(wrhs[:, DOUT : DOUT + 1], wa2_ps[:])

        # ---- xt chunks cast to bf16 (DVE): feeds s1b and the h/s2 matmuls
        xtb = []
        for c in range(XCH):
            t = const.tile([DIN, N // XCH], bf16, name=f"xtb{c}", tag=f"xtb{c}")
            nc.vector.tensor_copy(t[:], xt[c][:])
            xtb.append(t)

        # ---- s1b = wa1rep.T @ xT : [128, N] f32, straight from xT chunks
        s1b_sb = big.tile([PJ, N], f32, tag="s1b")
        with tc.tile_pool(name="s1b_psum", bufs=4, space="PSUM") as spool:
            for c in range(NCH):
                sl = slice(c * FCH, (c + 1) * FCH)
                s1b_ps = spool.tile([PJ, FCH], f32, name=f"s1bps{c}", tag="s1b_ps")
                nc.tensor.matmul(
                    s1b_ps[:], wa1rep[:], xtb[c][:],
                    start=True, stop=True,
                )
                nc.scalar.copy(s1b_sb[:, sl], s1b_ps[:])

        # ---- hcat regions (write-once, 4 tiles each) with ones columns
        hcat = [
            big.tile([PJ, HCG * HCS], bf16, name=f"hcat{g}", tag=f"hcat{g}")
            for g in range(NJT // HCG)
        ]
        s2g = [
            big.tile([PJ, HCG], f32, name=f"s2g{g}", tag=f"s2g{g}")
            for g in range(NJT // HCG)
        ]
        for g in range(NJT // HCG):
            h3 = hcat[g][:].rearrange("p (t s) -> p t s", s=HCS)
            nc.scalar.copy(
                h3[:, :, DOUT : DOUT + 1],
                ones_sb[:].broadcast_to([PJ, HCG])[:, :, None],
            )

        # ---- main pools ----
        mpool = ctx.enter_context(tc.tile_pool(name="mask", bufs=8))
        tpool = ctx.enter_context(tc.tile_pool(name="scores", bufs=3))
        ppool_e = ctx.enter_context(tc.tile_pool(name="probs", bufs=3))
        hpool = ctx.enter_context(tc.tile_pool(name="h_psum", bufs=2, space="PSUM"))
        num_pool = ctx.enter_context(
            tc.tile_pool(name="num_psum", bufs=1, space="PSUM")
        )
        numT_ps = num_pool.tile([DOUT + 1, N], f32, tag="numt")

        # ---- h/s2 tiles: per group of 4, one matmul per tile into a shared
        # psum bank tile, then ONE strided ACT drain for h and one for s2.
        def emit_hgroup(g):
            h_ps = hpool.tile([PJ, HCG * (DOUT + 1)], f32, tag="hps")
            for k in range(HCG):
                jt = g * HCG + k
                c, off = jt // XCH, (jt % XCH) * PJ
                nc.tensor.matmul(
                    h_ps[:, k * (DOUT + 1) : (k + 1) * (DOUT + 1)],
                    xtb[c][:, off : off + PJ],
                    wrhs[:],
                    start=True, stop=True,
                )
            h4 = h_ps[:].rearrange("p (t s) -> p t s", s=DOUT + 1)
            hc3 = hcat[g][:].rearrange("p (t s) -> p t s", s=HCS)
            nc.scalar.copy(hc3[:, :, :DOUT], h4[:, :, :DOUT])
            nc.scalar.copy(s2g[g][:], h4[:, :, DOUT])

        emit_hgroup(0)

        # ---- mask stream: pre-emit ALL tile DMAs so the sync engine can run
        # ahead of the compute loop (probes gate the first 8 behind xT; tiles
        # 8-15 wait on buffer rotation WAR deps automatically).
        mbs = []
        for jt in range(NJT):
            mb = mpool.tile([PJ, N], u8, name=f"mb{jt}", tag="mb")
            if jt < 8:
                nc.vector.tensor_copy(mb[0:1, 0:1], xt[XCH - 1][0:1, 0:1])
            with tc.tile_wait_until(0.0010 + 0.00005 * jt):
                nc.sync.dma_start(mb[:], mk[jt * PJ : (jt + 1) * PJ, :])
            mbs.append(mb)

        emit_hgroup(1)

        # ---- main loop over j-tiles ----
        for jt in range(NJT):
            g, k = jt // HCG, jt % HCG
            if jt in (HCG, 2 * HCG) :
                emit_hgroup(g + 1)

            mb = mbs[jt]
            t_sb = tpool.tile([PJ, N], f32, tag="t")
            nc.vector._custom_dve(
                gat_op,
                out=t_sb[:],
                in0=s1b_sb[:],
                in1=mb[:],
                s0=s2g[g][:, k : k + 1],
                s1=NEG_BIG,
                imm2=5.0,
            )

            p_sb = ppool_e.tile([PJ, N], bf16, tag="p")
            nc.scalar.activation(p_sb[:], t_sb[:], AF.Exp, scale=0.2)

            lhsT = hcat[g][:, k * HCS : k * HCS + DOUT + 1]
            for c in range(NCH):
                sl = slice(c * FCH, (c + 1) * FCH)
                nc.tensor.matmul(
                    numT_ps[:, sl], lhsT, p_sb[:, sl],
                    start=(jt == 0), stop=(jt == NJT - 1),
                )

        # ---- epilogue: per 512-chunk: drain, transpose, divide, store ----
        epool = ctx.enter_context(tc.tile_pool(name="epi", bufs=2))
        etr_pool = ctx.enter_context(
            tc.tile_pool(name="epi_psum", bufs=2, space="PSUM")
        )
        out_pool = ctx.enter_context(tc.tile_pool(name="out", bufs=1))

        out_sb = out_pool.tile([PJ, NJT * DOUT], f32, tag="out")
        out_3d = out.rearrange("(t p) d -> p t d", p=PJ)
        GW = EPI_GRP * (DOUT + 1)
        for g in range(NJT // EPI_GRP):
            csl = slice(g * FCH, (g + 1) * FCH)
            numc = epool.tile([DOUT + 1, FCH], f32, tag="numc")
            nc.scalar.copy(numc[:], numT_ps[:, csl])

            tr_ps = etr_pool.tile([PJ, GW], f32, tag="tr")
            for k in range(EPI_GRP):
                isl = slice(k * PJ, (k + 1) * PJ)
                nc.tensor.transpose(
                    tr_ps[:, k * (DOUT + 1) : (k + 1) * (DOUT + 1)],
                    numc[:, isl],
                    ident[: DOUT + 1, : DOUT + 1],
                )
            tr_sb = epool.tile([PJ, GW], f32, tag="tr_sb")
            nc.scalar.copy(tr_sb[:], tr_ps[:])

            tr3 = tr_sb[:].rearrange("p (k s) -> p k s", s=DOUT + 1)
            recip = epool.tile([PJ, EPI_GRP], f32, tag="recip")
            nc.vector.reciprocal(recip[:], tr3[:, :, DOUT])
            for k in range(EPI_GRP):
                it = g * EPI_GRP + k
                nc.vector.tensor_scalar_mul(
                    out_sb[:, it * DOUT : (it + 1) * DOUT],
                    tr3[:, k, :DOUT],
                    recip[:, k : k + 1],
                )
            nc.sync.dma_start(
                out_3d[:, g * EPI_GRP : (g + 1) * EPI_GRP, :],
                out_sb[:, g * EPI_GRP * DOUT : (g + 1) * EPI_GRP * DOUT].rearrange(
                    "p (t d) -> p t d", d=DOUT
                ),
            )

    nc.compile()
    return nc


def _prep_inputs(x, adj, W, a):
    xT = np.ascontiguousarray(np.transpose(x, (0, 2, 1)), dtype=np.float32)
    mask = np.ascontiguousarray(adj.T.astype(np.uint8))
    a = np.asarray(a, dtype=np.float32)
    a1 = np.ascontiguousarray(a[:DOUT].reshape(DOUT, 1))
    a2 = np.ascontiguousarray(a[DOUT:].reshape(DOUT, 1))
    W = np.ascontiguousarray(np.asarray(W, dtype=np.float32))
    in_maps = []
    for b in range(NCORES):
        in_maps.append(
            {
                "xt": xT[b],
                "mask": mask,
                "w": W,
                "a1": a1,
                "a2": a2,
            }
        )
    return in_maps


def kernel(x, adj, W, a):
    global _COMPILED, LAST_RESULT
    from concourse import bass_utils

    x = np.asarray(x)
    adj = np.asarray(adj)
    assert x.shape == (B, N, DIN) and adj.shape == (N, N)

    if _COMPILED is None:
        _COMPILED = _build_nc()
    nc = _COMPILED

    in_maps = _prep_inputs(x, adj, W, a)
    res = bass_utils.run_bass_kernel_spmd(
        nc,
        in_maps,
        core_ids=list(range(NCORES)),
        trace=bool(int(os.environ.get("GAT_TRACE", "0"))),
    )
    LAST_RESULT = res
    out = np.stack([res.results[c]["out"] for c in range(NCORES)], axis=0)
    return out.astype(np.float32)


# revision 9
# speedup vs baseline: 1.1903x; 1.0267x over previous
"""GAT layer (nn_GATLayer) on 8 Trainium2 NeuronCores.

Math (per batch b):
    h   = x @ W                      [N, D]
    s1  = h @ a1   (free-dim i)      [N]
    s2  = h @ a2   (partition j)     [N]
    e   = lrelu(s1_i + s2_j)  masked by adj[i, j], softmax over j
    out = attn @ h

Device formulation (per core = one batch element), in [p=j, f=i] layout:
    t[j, i]  = select(A[j, i] > 0, max(y, 5y), -1e9),  y = s1[i] + s2[j]
               (custom DVE op; A is the uint8 adjacency -> 4MB/core DMA)
    p[j, i]  = exp(0.2 * t)                      (ACT, bf16 out)
    numT[d, i] = sum_j h_cat[j, d] * p[j, i],    h_cat = [h | ones]  (bf16)
    out[i, d]  = numT[d, i] / numT[64, i]

Sharding: data-parallel over batch B=8 across the 8 cores. Host prep:
x[b] transposed to xT [64, 2048] (4 column chunks on 4 dispatch engines);
mask = adj.T as uint8 (shared); W and W.T both shipped (16KB, kills the
on-device transpose from the wa1/wa2 chain).

The steady loop is DVE-bound (custom score op, 2048 cols x ~1.15ns); the
prologue is arranged so the first score op can start as soon as s1b's four
chunks are drained, and the last tile is split into 4 column chunks so the
exp/matmul tail pipelines instead of serializing.
"""

import os
import sys

sys.path.insert(0, "/opt/trn_rl_repo")

import numpy as np

B, N, DIN, DOUT = 8, 2048, 64, 64
NCORES = 8
PJ = 128              # j-tile partition size
NJT = N // PJ         # 16 j-tiles
FCH = 512             # psum bank chunk (fp32)
NCH = N // FCH        # 4 chunks of the free dim
XCH = 4               # xT column chunks (parallel DMA queues)
NEG_BIG = -1.0e9
HCG = 4               # h/s2 tiles per psum group / hcat region
HCS = 66              # hcat stride: 64 h cols + 1 ones col + 1 pad
EPI_GRP = 4           # epilogue transposes packed per psum bank tile

_GAT_OP = None
_COMPILED = None
LAST_RESULT = None    # BassKernelResults from the last run (for test.py)


def _register_gat_op():
    """Fused score op: out = select(Src1 > 0, max(y, y*imm2), C1), y = Src0+C0.

    in0 = s1 broadcast [128, N] (f32), s0 = s2 per-partition [128, 1] (f32),
    in1 = adjacency tile [128, N] (uint8 0/1), s1 = -1e9, imm2 = 5.0.
    lrelu(x) = 0.2*max(5x, x); exp(0.2 * -1e9) -> 0 for masked entries.
    """
    global _GAT_OP
    if _GAT_OP is not None:
        return _GAT_OP
    from concourse.dve_ops import (
        OPS,
        CUSTOM_DVE_SPECS,
        DveOp,
        _SUB_OPCODE_FOR_NAME,
    )
    from concourse.dve_spec import (
        Spec, Src0, Src1, C0, C1, C2, Zero, maxx, select, lower, _has_src1,
    )
    from concourse.dve_uop import DveOpSpec

    name = "GAT_SCORE_U8_ANT"
    if name in _SUB_OPCODE_FOR_NAME:
        _GAT_OP = next(op for op in OPS if op.name == name)
        return _GAT_OP

    _y = Src0 + C0
    body = select(Src1 > Zero, maxx(_y, _y * C2), C1)

    def _ref(in0, in1, s0, s1, imm2):
        y = in0.astype(np.float32) + s0
        t = np.maximum(y, y * imm2)
        return np.where(in1.astype(np.float32) > 0.0, t, s1).astype(np.float32)

    spec = Spec(body=body, reference=_ref)
    row = max(_SUB_OPCODE_FOR_NAME.values()) + 1
    assert row < 0x20
    _SUB_OPCODE_FOR_NAME[name] = row
    shas = {}
    for ver in ("v3", "v4"):
        tmp = DveOpSpec(
            name=name, opcode=row, uops=lower(spec, ver=ver), rd1_en=_has_src1(spec)
        )
        shas[ver] = tmp.sha(ver)
    op = DveOp(name, spec, subdim=False, uops_sha=shas)
    OPS.append(op)
    CUSTOM_DVE_SPECS[name] = spec
    _GAT_OP = op
    return op


def _build_nc():
    """Build the Bass module (shared SPMD program for all 8 cores)."""
    from contextlib import ExitStack

    import concourse.bass as bass
    import concourse.tile as tile
    from concourse import bacc, masks, mybir

    gat_op = _register_gat_op()

    f32 = mybir.dt.float32
    bf16 = mybir.dt.bfloat16
    u8 = mybir.dt.uint8
    AF = mybir.ActivationFunctionType

    nc = bacc.Bacc("TRN2", target_bir_lowering=False, debug=False, num_devices=NCORES)

    xT = nc.dram_tensor("xt", [DIN, N], f32, kind="ExternalInput").ap()
    mk = nc.dram_tensor("mask", [N, N], u8, kind="ExternalInput").ap()
    w = nc.dram_tensor("w", [DIN, DOUT], f32, kind="ExternalInput").ap()
    wt = nc.dram_tensor("wt", [DOUT, DIN], f32, kind="ExternalInput").ap()
    a1 = nc.dram_tensor("a1", [DOUT, 1], f32, kind="ExternalInput").ap()
    a2 = nc.dram_tensor("a2", [DOUT, 1], f32, kind="ExternalInput").ap()
    out = nc.dram_tensor("out", [N, DOUT], f32, kind="ExternalOutput").ap()

    with ExitStack() as ctx:
        tc = ctx.enter_context(tile.TileContext(nc))

        const = ctx.enter_context(tc.tile_pool(name="const", bufs=1))
        big = ctx.enter_context(tc.tile_pool(name="big", bufs=1))

        # ---- input DMAs: xT chunks on 4 different dispatch engines (their
        # queue rings run in parallel); w/wt/a1/a2 from gpsimd afterwards.
        xt = []
        xt_engines = [nc.sync, nc.scalar, nc.gpsimd, nc.sync]
        for c in range(XCH):
            t = const.tile([DIN, N // XCH], f32, name=f"xt{c}", tag=f"xt{c}")
            xt_engines[c].dma_start(t[:], xT[:, c * (N // XCH) : (c + 1) * (N // XCH)])
            xt.append(t)
        w_dma = const.tile([DIN, DOUT], f32, tag="w0")
        nc.gpsimd.dma_start(w_dma[:], w)
        wt_dma = const.tile([DOUT, DIN], f32, tag="wt0")
        nc.gpsimd.dma_start(wt_dma[:], wt)
        a1_dma = const.tile([DOUT, 1], f32, tag="a10")
        nc.gpsimd.dma_start(a1_dma[:], a1)
        a2_dma = const.tile([DOUT, 1], f32, tag="a20")
        nc.gpsimd.dma_start(a2_dma[:], a2)

        ones_sb = const.tile([PJ, 1], bf16, tag="ones")
        nc.vector.memset(ones_sb[:], 1.0)

        # ---- wa1 = W @ a1 (replicated bf16), wa2 = W @ a2; wrhs = [W | wa2]
        a1_sb = const.tile([DOUT, 1], f32, tag="a1")
        nc.scalar.copy(a1_sb[:], a1_dma[:])
        a2_sb = const.tile([DOUT, 1], f32, tag="a2")
        nc.scalar.copy(a2_sb[:], a2_dma[:])

        wrhs = const.tile([DIN, DOUT + 1], bf16, tag="wrhs")
        nc.scalar.copy(wrhs[:, :DOUT], w_dma[:])

        # psum pools: hpool and num_pool own their banks from the start so
        # the h-group matmuls never wait on recycled s1b banks.
        hpool = ctx.enter_context(tc.tile_pool(name="h_psum", bufs=1, space="PSUM"))
        num_pool = ctx.enter_context(
            tc.tile_pool(name="num_psum", bufs=1, space="PSUM")
        )

        with tc.tile_pool(name="wt_psum", bufs=1, space="PSUM") as wtpool:
            wa1_ps = wtpool.tile([DIN, 1], f32, tag="wa1_ps")
            nc.tensor.matmul(wa1_ps[:], wt_dma[:], a1_sb[:], start=True, stop=True)
            wa1rep = const.tile([DIN, PJ], bf16, tag="wa1rep")
            nc.scalar.copy(wa1rep[:], wa1_ps[:].broadcast_to([DIN, PJ]))
            wa2_ps = wtpool.tile([DIN, 1], f32, tag="wa2_ps")
            nc.tensor.matmul(wa2_ps[:], wt_dma[:], a2_sb[:], start=True, stop=True)
            nc.scalar.copy(wrhs[:, DOUT : DOUT + 1], wa2_ps[:])

        # ---- xt chunks cast to bf16: 2 on DVE, 2 on ACT (parallel)
        xtb = []
        for c in range(XCH):
            t = const.tile([DIN, N // XCH], bf16, name=f"xtb{c}", tag=f"xtb{c}")
            if c < 2:
                nc.vector.tensor_copy(t[:], xt[c][:])
            else:
                nc.scalar.copy(t[:], xt[c][:])
            xtb.append(t)

        # ---- hcat regions (write-once, 4 tiles each) + per-group s2 cols
        hcat = [
            big.tile([PJ, HCG * HCS], bf16, name=f"hcat{g}", tag=f"hcat{g}")
            for g in range(NJT // HCG)
        ]
        s2g = [
            big.tile([PJ, HCG], f32, name=f"s2g{g}", tag=f"s2g{g}")
            for g in range(NJT // HCG)
        ]
        for g in range(NJT // HCG):
            h3 = hcat[g][:].rearrange("p (t s) -> p t s", s=HCS)
            nc.vector.tensor_copy(
                h3[:, :, DOUT : DOUT + 1],
                ones_sb[:].broadcast_to([PJ, HCG])[:, :, None],
            )

        def emit_hgroup(g):
            h_ps = hpool.tile([PJ, HCG * (DOUT + 1)], f32, tag="hps")
            for k in range(HCG):
                jt = g * HCG + k
                c, off = jt // XCH, (jt % XCH) * PJ
                nc.tensor.matmul(
                    h_ps[:, k * (DOUT + 1) : (k + 1) * (DOUT + 1)],
                    xtb[c][:, off : off + PJ],
                    wrhs[:],
                    start=True, stop=True,
                )
            h4 = h_ps[:].rearrange("p (t s) -> p t s", s=DOUT + 1)
            hc3 = hcat[g][:].rearrange("p (t s) -> p t s", s=HCS)
            nc.scalar.copy(hc3[:, :, :DOUT], h4[:, :, :DOUT])
            nc.scalar.copy(s2g[g][:], h4[:, :, DOUT])

        # group 0 first on PE: it unblocks the first score op's s2 column
        emit_hgroup(0)

        # ---- s1b = wa1rep.T @ xT : [128, N] f32 (3 rotating psum banks)
        s1b_sb = big.tile([PJ, N], f32, tag="s1b")
        with tc.tile_pool(name="s1b_psum", bufs=3, space="PSUM") as spool:
            for c in range(NCH):
                sl = slice(c * FCH, (c + 1) * FCH)
                s1b_ps = spool.tile([PJ, FCH], f32, name=f"s1bps{c}", tag="s1b_ps")
                nc.tensor.matmul(
                    s1b_ps[:], wa1rep[:], xtb[c][:],
                    start=True, stop=True,
                )
                nc.scalar.copy(s1b_sb[:, sl], s1b_ps[:])

        # ---- mask stream: pre-emit ALL tile DMAs so the sync engine runs
        # ahead of the compute loop (probes gate the first 8 behind xT; tiles
        # 8-15 wait on buffer-rotation WAR deps automatically).
        mpool = ctx.enter_context(tc.tile_pool(name="mask", bufs=8))
        mbs = []
        for jt in range(NJT):
            mb = mpool.tile([PJ, N], u8, name=f"mb{jt}", tag="mb")
            if jt < 8:
                nc.vector.tensor_copy(mb[0:1, 0:1], xt[XCH - 1][0:1, 0:1])
            with tc.tile_wait_until(0.0010 + 0.00005 * jt):
                nc.sync.dma_start(mb[:], mk[jt * PJ : (jt + 1) * PJ, :])
            mbs.append(mb)

        emit_hgroup(1)

        tpool = ctx.enter_context(tc.tile_pool(name="scores", bufs=3))
        ppool_e = ctx.enter_context(tc.tile_pool(name="probs", bufs=3))
        numT = [
            num_pool.tile([DOUT + 1, FCH], f32, name=f"numt{c}", tag=f"numt{c}")
            for c in range(NCH)
        ]

        # ---- main loop over j-tiles (last tile split in 4 column chunks so
        # the score/exp/matmul tail pipelines instead of serializing)
        for jt in range(NJT):
            g, k = jt // HCG, jt % HCG
            if jt in (HCG, 2 * HCG):
                emit_hgroup(g + 1)

            mb = mbs[jt]
            lhsT = hcat[g][:, k * HCS : k * HCS + DOUT + 1]
            t_sb = tpool.tile([PJ, N], f32, tag="t")
            p_sb = ppool_e.tile([PJ, N], bf16, tag="p")

            csls = [slice(0, N)] if jt < NJT - 1 else [
                slice(c * FCH, (c + 1) * FCH) for c in range(NCH)
            ]
            for csl in csls:
                nc.vector._custom_dve(
                    gat_op,
                    out=t_sb[:, csl],
                    in0=s1b_sb[:, csl],
                    in1=mb[:, csl],
                    s0=s2g[g][:, k : k + 1],
                    s1=NEG_BIG,
                    imm2=5.0,
                )
                nc.scalar.activation(p_sb[:, csl], t_sb[:, csl], AF.Exp, scale=0.2)

            for c in range(NCH):
                sl = slice(c * FCH, (c + 1) * FCH)
                nc.tensor.matmul(
                    numT[c][:], lhsT, p_sb[:, sl],
                    start=(jt == 0), stop=(jt == NJT - 1),
                )

        # ---- epilogue: per 512-chunk: drain, transpose, divide, store ----
        ident0 = const.tile([PJ, PJ], f32, tag="ident0")
        masks.make_identity(nc, ident0[:])
        ident = const.tile([PJ, PJ], f32, tag="ident")
        nc.scalar.copy(ident[:], ident0[:])

        epool = ctx.enter_context(tc.tile_pool(name="epi", bufs=2))
        etr_pool = ctx.enter_context(
            tc.tile_pool(name="epi_psum", bufs=2, space="PSUM")
        )
        out_pool = ctx.enter_context(tc.tile_pool(name="out", bufs=1))

        out_sb = out_pool.tile([PJ, NJT * DOUT], f32, tag="out")
        out_3d = out.rearrange("(t p) d -> p t d", p=PJ)
        out_engines = [nc.sync, nc.scalar, nc.gpsimd, nc.sync]
        GW = EPI_GRP * (DOUT + 1)
        for g in range(NJT // EPI_GRP):
            numc = epool.tile([DOUT + 1, FCH], f32, tag="numc")
            nc.scalar.copy(numc[:], numT[g][:])

            tr_ps = etr_pool.tile([PJ, GW], f32, tag="tr")
            for k in range(EPI_GRP):
                isl = slice(k * PJ, (k + 1) * PJ)
                nc.tensor.transpose(
                    tr_ps[:, k * (DOUT + 1) : (k + 1) * (DOUT + 1)],
                    numc[:, isl],
                    ident[: DOUT + 1, : DOUT + 1],
                )
            tr_sb = epool.tile([PJ, GW], f32, tag="tr_sb")
            nc.scalar.copy(tr_sb[:], tr_ps[:])

            tr3 = tr_sb[:].rearrange("p (k s) -> p k s", s=DOUT + 1)
            recip = epool.tile([PJ, EPI_GRP], f32, tag="recip")
            nc.vector.reciprocal(recip[:], tr3[:, :, DOUT])
            for k in range(EPI_GRP):
                it = g * EPI_GRP + k
                nc.vector.tensor_scalar_mul(
                    out_sb[:, it * DOUT : (it + 1) * DOUT],
                    tr3[:, k, :DOUT],
                    recip[:, k : k + 1],
                )
            out_engines[g].dma_start(
                out_3d[:, g * EPI_GRP : (g + 1) * EPI_GRP, :],
                out_sb[:, g * EPI_GRP * DOUT : (g + 1) * EPI_GRP * DOUT].rearrange(
                    "p (t d) -> p t d", d=DOUT
                ),
            )

    nc.compile()
    return nc


def _prep_inputs(x, adj, W, a):
    xT = np.ascontiguousarray(np.transpose(x, (0, 2, 1)), dtype=np.float32)
    mask = np.ascontiguousarray(adj.T.astype(np.uint8))
    a = np.asarray(a, dtype=np.float32)
    a1 = np.ascontiguousarray(a[:DOUT].reshape(DOUT, 1))
    a2 = np.ascontiguousarray(a[DOUT:].reshape(DOUT, 1))
    W = np.ascontiguousarray(np.asarray(W, dtype=np.float32))
    Wt = np.ascontiguousarray(W.T)
    in_maps = []
    for b in range(NCORES):
        in_maps.append(
            {
                "xt": xT[b],
                "mask": mask,
                "w": W,
                "wt": Wt,
                "a1": a1,
                "a2": a2,
            }
        )
    return in_maps


def kernel(x, adj, W, a):
    global _COMPILED, LAST_RESULT
    from concourse import bass_utils

    x = np.asarray(x)
    adj = np.asarray(adj)
    assert x.shape == (B, N, DIN) and adj.shape == (N, N)

    if _COMPILED is None:
        _COMPILED = _build_nc()
    nc = _COMPILED

    in_maps = _prep_inputs(x, adj, W, a)
    res = bass_utils.run_bass_kernel_spmd(
        nc,
        in_maps,
        core_ids=list(range(NCORES)),
        trace=bool(int(os.environ.get("GAT_TRACE", "0"))),
    )
    LAST_RESULT = res
    out = np.stack([res.results[c]["out"] for c in range(NCORES)], axis=0)
    return out.astype(np.float32)
